# revision 16
# baseline (speedup 1.0000x reference)
"""MemoryCrossAttention Trainium2 Bass kernel (v2, bf16 pipeline).

8-core data-parallel over query rows: core c handles batch c//2, row-half
c%2 (R=2048 rows). All matmuls run in bf16 (FWL weight loads, f32 PSUM
accumulation). Attention for head pair p is interleaved between projection
units (Q htp / G htp) so the PE stays dense and HAM-warm. PSUM budget:
4 banks proj (double-buffered groups) + 2 scores + 1 denom + 1 attn.
Softmax denominators use reciprocal_approx_fast + a DRAM stride-0
broadcast; attention output is normalized post-eviction on DVE.
"""
from contextlib import ExitStack

import numpy as np

import concourse.bass as bass
import concourse.tile as tile
from concourse import mybir
from concourse.bass_utils import run_bass_kernel_spmd

F32 = mybir.dt.float32
BF16 = mybir.dt.bfloat16
P = 128

_H, _NH, _HD, _M = 2048, 16, 128, 256
_B, _L = 4, 4096
_R = 2048            # rows per core
_NCORES = 8
_EPS = 1e-6
_KT = _H // P        # 16 contraction tiles
_MT = _M // P        # 2
_NHTP = _NH // 2     # 8 head pairs
_SCALE = _HD ** -0.5


def _bcast_ap(ap, p=P):
    return bass.AP(tensor=ap.tensor, offset=ap.offset, ap=[[0, p]] + ap.ap)


def build(nc, MP):
    H, NH, R, KT, NHTP = _H, _NH, _R, _KT, _NHTP
    M, MT = MP, MP // P
    LH = R // 1024       # 2 row-halves (1024) per R
    LQ = R // 512        # 4 512-chunks

    xT = nc.dram_tensor("xT", [H, R], F32, kind="ExternalInput")
    memTb = nc.dram_tensor("memTb", [P, KT * M], BF16, kind="ExternalInput")
    maskb = nc.dram_tensor("maskb", [P, MT], F32, kind="ExternalInput")
    wqTb = nc.dram_tensor("wqTb", [NHTP, P, KT * 256], BF16, kind="ExternalInput")
    wgTb = nc.dram_tensor("wgTb", [NHTP, P, KT * 256], BF16, kind="ExternalInput")
    woTb = nc.dram_tensor("woTb", [NHTP, P, KT * 256], BF16, kind="ExternalInput")
    wkTb = nc.dram_tensor("wkTb", [4, P, KT * 512], BF16, kind="ExternalInput")
    wvTb = nc.dram_tensor("wvTb", [4, P, KT * 512], BF16, kind="ExternalInput")
    outT = nc.dram_tensor("outT", [H, R], F32, kind="ExternalOutput")

    with tile.TileContext(nc) as tc, ExitStack() as ctx:
        dram = ctx.enter_context(tc.tile_pool(name="dram", bufs=1, space="DRAM"))
        aspill = dram.tile([NH, P, R], BF16)
        gspill = dram.tile([NH, P, R], BF16)
        rs_scr = dram.tile([R], BF16)
        rd_scr = dram.tile([NH, R], BF16)

        const = ctx.enter_context(tc.tile_pool(name="const", bufs=1))
        ones_f32 = const.tile([P, 1], F32)
        nc.vector.memset(ones_f32, 1.0)
        ones_bf = const.tile([P, 1], BF16)
        nc.vector.tensor_copy(ones_bf, ones_f32)
        eps_sb = const.tile([1, 1], F32)
        nc.vector.memset(eps_sb, _EPS)
        mask_sb = const.tile([P, MT], F32)
        nc.sync.dma_start(out=mask_sb, in_=maskb[:])

        # persistent SBUF tensors
        kv = ctx.enter_context(tc.tile_pool(name="kv", bufs=1))
        kT_sb = kv.tile([P, NH, M], BF16)       # [d, h, m]
        vmd_sb = kv.tile([P, MT, H], BF16)      # [m, mt, h*d]
        sbc_p = ctx.enter_context(tc.tile_pool(name="sbc", bufs=1))
        s_bc = sbc_p.tile([P, R], BF16)

        xbfp = ctx.enter_context(tc.tile_pool(name="xbfp", bufs=1))
        xbf = xbfp.tile([P, KT, R], BF16)

        wst = ctx.enter_context(tc.tile_pool(name="wst", bufs=3))

        phaseA = ExitStack()
        memp = phaseA.enter_context(tc.tile_pool(name="memp", bufs=1))
        mem_sb = memp.tile([P, KT, M], BF16)
        wkvp = phaseA.enter_context(tc.tile_pool(name="wkv", bufs=3))

        # ---- DMA issue order: mem, wk0, then x interleaved with weights ----
        nc.sync.dma_start(out=mem_sb, in_=memTb[:])
        wk_t, wv_t = {}, {}

        def alloc_wkv(d, i, src_t):
            d[i] = wkvp.tile([P, KT * 512], BF16, name="wkv")
            nc.sync.dma_start(out=d[i], in_=src_t[i])

        alloc_wkv(wk_t, 0, wkTb)
        xfp = phaseA.enter_context(tc.tile_pool(name="xf", bufs=3))
        x2p = phaseA.enter_context(tc.tile_pool(name="x2", bufs=2))
        x2s = []
        for kt in range(KT):
            xf = xfp.tile([P, R], F32, name="xf")
            nc.sync.dma_start(out=xf, in_=xT[kt * P:(kt + 1) * P, :])
            if kt in (3, 6, 9):
                alloc_wkv(wk_t, kt // 3, wkTb)
            if kt in (10, 11, 12, 13):
                alloc_wkv(wv_t, kt - 10, wvTb)
            nc.vector.tensor_copy(xbf[:, kt, :], xf)
            x2 = x2p.tile([P, R], BF16, name="x2")
            nc.vector.tensor_mul(x2, xf, xf)
            x2s.append(x2)

        # ---- Phase A PE work: K rounds interleaved with ssq partition-sums --
        with tc.tile_pool(name="kps", bufs=1, space="PSUM") as kps, \
             tc.tile_pool(name="ssqp", bufs=1, space="PSUM") as ssqp, \
             tc.tile_pool(name="sp", bufs=1) as sp:
            ssq = ssqp.tile([1, R], F32)

            def emit_ssq(kts):
                for kt in kts:
                    for lq in range(LQ):
                        nc.tensor.matmul(
                            ssq[0:1, lq * 512:(lq + 1) * 512], ones_bf,
                            x2s[kt][:, lq * 512:(lq + 1) * 512],
                            start=(kt == 0), stop=(kt == KT - 1))

            for rnd in range(4):
                kpsum = kps.tile([P, 2048], F32, name="kpsum")
                for kt in range(KT):
                    for hh in range(4):
                        nc.tensor.matmul(
                            kpsum[:, hh * 512:hh * 512 + M],
                            wk_t[rnd][:, kt * 512 + hh * P:
                                      kt * 512 + (hh + 1) * P],
                            mem_sb[:, kt, :],
                            start=(kt == 0), stop=(kt == KT - 1))
                for hh in range(4):
                    nc.scalar.copy(kT_sb[:, rnd * 4 + hh, :],
                                   kpsum[:, hh * 512:hh * 512 + M])
                emit_ssq(range(rnd * 4, rnd * 4 + 4))

            s_sb = sp.tile([1, R], F32)
            nc.scalar.activation(
                s_sb, ssq[0:1, :], mybir.ActivationFunctionType.Sqrt,
                bias=eps_sb, scale=1.0 / H)
            nc.vector.reciprocal_approx_fast(s_sb, s_sb)
            nc.gpsimd.dma_start(out=rs_scr[:], in_=s_sb[0:1, :])


        # V proj after K/ssq psum pools close
        with tc.tile_pool(name="vps", bufs=2, space="PSUM") as vps:
            for dc in range(4):
                vpsum = vps.tile([P, 1024], F32, name="vpsum")
                for kt in range(KT):
                    for mt in range(MT):
                        nc.tensor.matmul(
                            vpsum[:, mt * 512:(mt + 1) * 512],
                            mem_sb[:, kt, mt * P:(mt + 1) * P],
                            wv_t[dc][:, kt * 512:(kt + 1) * 512],
                            start=(kt == 0), stop=(kt == KT - 1))
                for mt in range(MT):
                    nc.scalar.copy(vmd_sb[:, mt, dc * 512:(dc + 1) * 512],
                                   vpsum[:, mt * 512:(mt + 1) * 512])

        phaseA.close()

        # prefetch first two Q weight tiles (ahead of the blocking s_bc wait)
        wts = {}
        for u in (0, 1):
            wts[u] = wst.tile([P, KT * 256], BF16, name="wt")
            nc.sync.dma_start(out=wts[u], in_=wqTb[u])

        nc.gpsimd.dma_start(out=s_bc, in_=_bcast_ap(rs_scr[:]))
        for kt in range(KT):
            nc.vector.tensor_mul(xbf[:, kt, :], xbf[:, kt, :], s_bc)

        # ============ Main interleaved units ============
        with tc.tile_pool(name="pps", bufs=2, space="PSUM") as pps, \
             tc.tile_pool(name="gstg", bufs=2) as gstg:

            def emit_proj(u):
                htp = u % 8
                if u + 2 < 16:
                    wts[u + 2] = wst.tile([P, KT * 256], BF16, name="wt")
                    nc.sync.dma_start(out=wts[u + 2],
                                      in_=(wqTb if u + 2 < 8 else wgTb)[
                                          (u + 2) % 8])
                wt = wts.pop(u)
                if u < 8:
                    qt = qp.tile([P, 2, R], BF16, name="qt")
                    q_tiles[htp] = qt
                for h2 in range(2):
                    for rh in range(LH):
                        ppsum = pps.tile([P, 1024], F32, name="ppsum")
                        for kt in range(KT):
                            for lq in range(2):
                                sl = slice(rh * 1024 + lq * 512,
                                           rh * 1024 + (lq + 1) * 512)
                                nc.tensor.matmul(
                                    ppsum[:, lq * 512:(lq + 1) * 512],
                                    wt[:, kt * 256 + h2 * P:
                                       kt * 256 + (h2 + 1) * P],
                                    xbf[:, kt, sl],
                                    start=(kt == 0), stop=(kt == KT - 1))
                        osl = slice(rh * 1024, (rh + 1) * 1024)
                        if u < 8:
                            nc.scalar.copy(q_tiles[htp][:, h2, osl], ppsum)
                        else:
                            g = gstg.tile([P, 1024], BF16, name="g")
                            nc.scalar.activation(
                                g, ppsum,
                                mybir.ActivationFunctionType.Sigmoid)
                            nc.sync.dma_start(
                                out=gspill[htp * 2 + h2][:, osl], in_=g)

            with tc.tile_pool(name="qp", bufs=2) as qp, \
                 tc.tile_pool(name="sps", bufs=1, space="PSUM") as sps, \
                 tc.tile_pool(name="dps", bufs=1, space="PSUM") as dps, \
                 tc.tile_pool(name="aps", bufs=1, space="PSUM") as aps, \
                 tc.tile_pool(name="probs", bufs=3) as probsp, \
                 tc.tile_pool(name="aup", bufs=4) as aup, \
                 tc.tile_pool(name="asb", bufs=2) as asbp, \
                 tc.tile_pool(name="rdp", bufs=1) as rdp, \
                 tc.tile_pool(name="rbc", bufs=2) as rbcp:

                q_tiles = {}
                probs_t = {}
                au_t = {}
                rbc_t = {}

                def emit_scores_exp(pair):
                    qt = q_tiles[pair]
                    for hh in range(2):
                        h = pair * 2 + hh
                        pr = probsp.tile([P, MT, R], BF16, name="probs")
                        probs_t[h] = pr
                        for mt in range(MT):
                            for lh in range(LH):
                                spsum = sps.tile([P, 1024], F32, name="spsum")
                                for j in range(2):
                                    sl = slice((lh * 2 + j) * 512,
                                               (lh * 2 + j + 1) * 512)
                                    nc.tensor.matmul(
                                        spsum[:, j * 512:(j + 1) * 512],
                                        kT_sb[:, h, mt * P:(mt + 1) * P],
                                        qt[:, hh, sl], start=True, stop=True)
                                nc.scalar.activation(
                                    pr[:, mt, lh * 1024:(lh + 1) * 1024],
                                    spsum, mybir.ActivationFunctionType.Exp,
                                    bias=mask_sb[:, mt:mt + 1], scale=_SCALE)

                def emit_denom_attn(pair):
                    for hh in range(2):
                        h = pair * 2 + hh
                        pr = probs_t[h]
                        rden = rdp.tile([1, R], F32, name="rden")
                        for lq in range(LQ):
                            sl = slice(lq * 512, (lq + 1) * 512)
                            dpsum = dps.tile([1, 512], F32, name="dpsum")
                            for mt in range(MT):
                                nc.tensor.matmul(
                                    dpsum, ones_bf, pr[:, mt, sl],
                                    start=(mt == 0), stop=(mt == MT - 1))
                            nc.vector.reciprocal_approx_fast(rden[:, sl], dpsum)
                        nc.gpsimd.dma_start(out=rd_scr[h:h + 1, :],
                                            in_=rden[0:1, :])
                        rbc = rbcp.tile([P, R], BF16, name="rbc")
                        rbc_t[h] = rbc
                        nc.gpsimd.dma_start(out=rbc,
                                            in_=_bcast_ap(rd_scr[h, :]))
                        au = aup.tile([P, R], BF16, name="au")
                        au_t[h] = au
                        for lq in range(LQ):
                            sl = slice(lq * 512, (lq + 1) * 512)
                            apsum = aps.tile([P, 512], F32, name="apsum")
                            for mt in range(MT):
                                nc.tensor.matmul(
                                    apsum, vmd_sb[:, mt, h * P:(h + 1) * P],
                                    pr[:, mt, sl],
                                    start=(mt == 0), stop=(mt == MT - 1))
                            nc.scalar.copy(au[:, sl], apsum)
                        del probs_t[h]

                def emit_norm_spill(pair):
                    for hh in range(2):
                        h = pair * 2 + hh
                        attn_sb = asbp.tile([P, R], BF16, name="attn_sb")
                        nc.vector.tensor_mul(attn_sb, au_t[h], rbc_t[h])
                        nc.sync.dma_start(out=aspill[h], in_=attn_sb)
                        del au_t[h], rbc_t[h]

                for u in range(10):
                    pair = u - 1
                    if 0 <= pair < NHTP:
                        emit_scores_exp(pair)
                    emit_proj(u)
                    if 0 <= pair < NHTP:
                        emit_denom_attn(pair)
                        if pair >= 1:
                            emit_norm_spill(pair - 1)
                        if pair == NHTP - 1:
                            emit_norm_spill(pair)

            # attention pools closed: prefetch attn spills + first O weights
            at_stack = ExitStack()
            atp = at_stack.enter_context(tc.tile_pool(name="atp", bufs=1))
            at_sb = atp.tile([P, NH, R], BF16)
            wop = at_stack.enter_context(tc.tile_pool(name="wop", bufs=2))
            wo_ts = {}
            for i in (0, 1):
                wo_ts[i] = wop.tile([P, KT * 256], BF16, name="wo")
                nc.sync.dma_start(out=wo_ts[i], in_=woTb[i])
            for h in range(NH):
                nc.sync.dma_start(out=at_sb[:, h, :], in_=aspill[h])

            for u in range(10, 16):
                emit_proj(u)

            # ============ Phase D: O proj + gate ============
            with tc.tile_pool(name="gin", bufs=3) as ginp, \
                 tc.tile_pool(name="osb", bufs=2) as osbp:
                for htp in range(NHTP):
                    if htp + 2 < NHTP:
                        wo_ts[htp + 2] = wop.tile([P, KT * 256], BF16,
                                                  name="wo")
                        nc.sync.dma_start(out=wo_ts[htp + 2],
                                          in_=woTb[htp + 2])
                    wt = wo_ts.pop(htp)
                    for h2 in range(2):
                        o = htp * 2 + h2
                        for rh in range(LH):
                            osl = slice(rh * 1024, (rh + 1) * 1024)
                            g_in = ginp.tile([P, 1024], BF16, name="g_in")
                            nc.sync.dma_start(out=g_in,
                                              in_=gspill[o][:, osl])
                            o_sb = osbp.tile([P, 1024], F32, name="o_sb")
                            opsum = pps.tile([P, 1024], F32, name="ppsum")
                            for kt in range(KT):
                                for lq in range(2):
                                    sl = slice(rh * 1024 + lq * 512,
                                               rh * 1024 + (lq + 1) * 512)
                                    nc.tensor.matmul(
                                        opsum[:, lq * 512:(lq + 1) * 512],
                                        wt[:, kt * 256 + h2 * P:
                                           kt * 256 + (h2 + 1) * P],
                                        at_sb[:, kt, sl],
                                        start=(kt == 0), stop=(kt == KT - 1))
                            nc.vector.tensor_mul(o_sb, opsum, g_in)
                            nc.sync.dma_start(
                                out=outT[o * P:(o + 1) * P, osl], in_=o_sb)
            at_stack.close()

    nc.compile()
    return nc


# ===================== host side =====================

def _bf16(a):
    import ml_dtypes
    return np.ascontiguousarray(a.astype(ml_dtypes.bfloat16))


def _pack_w_256(w_io):
    # w_io [in=H, out=H] -> [NHTP, 128, KT*256]; [htp, p, kt*256+c] =
    # w_io[kt*128+p, htp*256+c]
    return _bf16(np.ascontiguousarray(
        w_io.reshape(_KT, P, _NHTP, 256).transpose(2, 1, 0, 3)
        .reshape(_NHTP, P, _KT * 256)))


def _pack_w_512(w_io):
    # w_io [in=H, out=H] -> [4, 128, KT*512]
    return _bf16(np.ascontiguousarray(
        w_io.reshape(_KT, P, 4, 512).transpose(2, 1, 0, 3)
        .reshape(4, P, _KT * 512)))


_nc_cache = {}


def kernel(hidden_states, memory_tokens, memory_mask, norm_w,
           wq, wk, wv, wo, wg):
    import concourse.bacc as bacc

    hs = np.asarray(hidden_states, dtype=np.float32)
    mem = np.asarray(memory_tokens, dtype=np.float32)
    mask = np.asarray(memory_mask)
    norm_w = np.asarray(norm_w, dtype=np.float32)

    wq_n = (np.asarray(wq, dtype=np.float32) * norm_w[None, :]).T
    wg_n = (np.asarray(wg, dtype=np.float32) * norm_w[None, :]).T
    shared = {
        "wqTb": _pack_w_256(np.ascontiguousarray(wq_n)),
        "wgTb": _pack_w_256(np.ascontiguousarray(wg_n)),
        "woTb": _pack_w_256(np.ascontiguousarray(
            np.asarray(wo, dtype=np.float32).T)),
        "wkTb": _pack_w_512(np.ascontiguousarray(
            np.asarray(wk, dtype=np.float32).T)),
        "wvTb": _pack_w_512(np.ascontiguousarray(
            np.asarray(wv, dtype=np.float32).T)),
    }

    # compact memory tokens: drop masked tokens, pad to MP
    cnt_max = int(mask.sum(axis=1).max())
    MP = 128 if cnt_max <= 128 else 256
    MTP = MP // P

    in_maps = []
    for c in range(_NCORES):
        b, half = c // 2, c % 2
        inp = dict(shared)
        hs_slice = hs[b, half * _R:(half + 1) * _R, :]
        inp["xT"] = np.ascontiguousarray(hs_slice.T.astype(np.float32))
        idx = np.nonzero(mask[b])[0]
        mem_c = np.zeros((MP, _H), dtype=np.float32)
        mem_c[:len(idx)] = mem[b][idx]
        mv = np.full(MP, -50.0, dtype=np.float32)
        mv[:len(idx)] = 0.0
        inp["memTb"] = _bf16(np.ascontiguousarray(
            mem_c.T.reshape(_KT, P, MP).transpose(1, 0, 2)
            .reshape(P, _KT * MP)))
        inp["maskb"] = np.ascontiguousarray(mv.reshape(MTP, P).T)
        in_maps.append(inp)

    if _nc_cache.get(MP) is None:
        nc = bacc.Bacc(None, target_bir_lowering=False, debug=False)
        build(nc, MP)
        _nc_cache[MP] = nc
    nc = _nc_cache[MP]

    import os
    trace = os.environ.get("KERNEL_TRACE") == "1"
    res = run_bass_kernel_spmd(nc, in_maps, core_ids=list(range(_NCORES)),
                               trace=trace)
    kernel.last_result = res

    out = np.empty((_B, _L, _H), dtype=np.float32)
    for c in range(_NCORES):
        b, half = c // 2, c % 2
        out[b, half * _R:(half + 1) * _R, :] = res.results[c]["outT"].T
    return out


# revision 17
# speedup vs baseline: 1.2160x; 1.2160x over previous
"""MemoryCrossAttention Trainium2 Bass kernel (v2, bf16 pipeline).

8-core data-parallel over query rows: core c handles batch c//2, row-half
c%2 (R=2048 rows). All matmuls run in bf16 (FWL weight loads, f32 PSUM
accumulation). Attention for head pair p is interleaved between projection
units (Q htp / G htp) so the PE stays dense and HAM-warm. PSUM budget:
4 banks proj (double-buffered groups) + 2 scores + 1 denom + 1 attn.
Softmax denominators use reciprocal_approx_fast + a DRAM stride-0
broadcast; attention output is normalized post-eviction on DVE.
"""
from contextlib import ExitStack

import numpy as np

import concourse.bass as bass
import concourse.tile as tile
from concourse import mybir
from concourse.bass_utils import run_bass_kernel_spmd

F32 = mybir.dt.float32
BF16 = mybir.dt.bfloat16
P = 128

_H, _NH, _HD, _M = 2048, 16, 128, 256
_B, _L = 4, 4096
_R = 2048            # rows per core
_NCORES = 8
_EPS = 1e-6
_KT = _H // P        # 16 contraction tiles
_MT = _M // P        # 2
_NHTP = _NH // 2     # 8 head pairs
_SCALE = _HD ** -0.5


def _bcast_ap(ap, p=P):
    return bass.AP(tensor=ap.tensor, offset=ap.offset, ap=[[0, p]] + ap.ap)


def build(nc, MP):
    H, NH, R, KT, NHTP = _H, _NH, _R, _KT, _NHTP
    M, MT = MP, MP // P
    LH = R // 1024       # 2 row-halves (1024) per R
    LQ = R // 512        # 4 512-chunks

    xTb = nc.dram_tensor("xTb", [P, KT * R], BF16, kind="ExternalInput")
    memTb = nc.dram_tensor("memTb", [P, KT * M], BF16, kind="ExternalInput")
    maskb = nc.dram_tensor("maskb", [P, MT], F32, kind="ExternalInput")
    wqTb = nc.dram_tensor("wqTb", [NHTP, P, KT * 256], BF16, kind="ExternalInput")
    wgTb = nc.dram_tensor("wgTb", [NHTP, P, KT * 256], BF16, kind="ExternalInput")
    woTb = nc.dram_tensor("woTb", [NHTP, P, KT * 256], BF16, kind="ExternalInput")
    wkTb = nc.dram_tensor("wkTb", [4, P, KT * 512], BF16, kind="ExternalInput")
    wvTb = nc.dram_tensor("wvTb", [4, P, KT * 512], BF16, kind="ExternalInput")
    outT = nc.dram_tensor("outT", [H, R], F32, kind="ExternalOutput")

    with tile.TileContext(nc) as tc, ExitStack() as ctx:
        dram = ctx.enter_context(tc.tile_pool(name="dram", bufs=1, space="DRAM"))
        aspill = dram.tile([NH, P, R], BF16)
        gspill = dram.tile([NH, P, R], BF16)
        rs_scr = dram.tile([R], BF16)
        rd_scr = dram.tile([NH, R], BF16)

        const = ctx.enter_context(tc.tile_pool(name="const", bufs=1))
        ones_f32 = const.tile([P, 1], F32)
        nc.vector.memset(ones_f32, 1.0)
        ones_bf = const.tile([P, 1], BF16)
        nc.vector.tensor_copy(ones_bf, ones_f32)
        eps_sb = const.tile([1, 1], F32)
        nc.vector.memset(eps_sb, _EPS)
        mask_sb = const.tile([P, MT], F32)
        nc.sync.dma_start(out=mask_sb, in_=maskb[:])

        # persistent SBUF tensors
        kv = ctx.enter_context(tc.tile_pool(name="kv", bufs=1))
        kT_sb = kv.tile([P, NH, M], BF16)       # [d, h, m]
        vmd_sb = kv.tile([P, MT, H], BF16)      # [m, mt, h*d]
        sbc_p = ctx.enter_context(tc.tile_pool(name="sbc", bufs=1))
        s_bc = sbc_p.tile([P, R], BF16)

        xbfp = ctx.enter_context(tc.tile_pool(name="xbfp", bufs=1))
        xbf = xbfp.tile([P, KT, R], BF16)

        wst = ctx.enter_context(tc.tile_pool(name="wst", bufs=3))

        phaseA = ExitStack()
        memp = phaseA.enter_context(tc.tile_pool(name="memp", bufs=1))
        mem_sb = memp.tile([P, KT, M], BF16)
        wkvp = phaseA.enter_context(tc.tile_pool(name="wkv", bufs=3))

        # ---- DMA issue order: mem, wk0, then x interleaved with weights ----
        nc.sync.dma_start(out=mem_sb, in_=memTb[:])
        wk_t, wv_t = {}, {}

        def alloc_wkv(d, i, src_t):
            d[i] = wkvp.tile([P, KT * 512], BF16, name="wkv")
            nc.sync.dma_start(out=d[i], in_=src_t[i])

        alloc_wkv(wk_t, 0, wkTb)
        x2p = phaseA.enter_context(tc.tile_pool(name="x2", bufs=2))
        x2s = []
        for kt in range(KT):
            nc.sync.dma_start(out=xbf[:, kt, :],
                              in_=xTb[:, kt * R:(kt + 1) * R])
            if kt in (3, 6, 9):
                alloc_wkv(wk_t, kt // 3, wkTb)
            if kt in (10, 11, 12, 13):
                alloc_wkv(wv_t, kt - 10, wvTb)
            x2 = x2p.tile([P, R], BF16, name="x2")
            nc.vector.tensor_mul(x2, xbf[:, kt, :], xbf[:, kt, :])
            x2s.append(x2)

        # ---- Phase A PE work: K rounds interleaved with ssq partition-sums --
        with tc.tile_pool(name="kps", bufs=1, space="PSUM") as kps, \
             tc.tile_pool(name="ssqp", bufs=1, space="PSUM") as ssqp, \
             tc.tile_pool(name="sp", bufs=1) as sp:
            ssq = ssqp.tile([1, R], F32)

            def emit_ssq(kts):
                for kt in kts:
                    for lq in range(LQ):
                        nc.tensor.matmul(
                            ssq[0:1, lq * 512:(lq + 1) * 512], ones_bf,
                            x2s[kt][:, lq * 512:(lq + 1) * 512],
                            start=(kt == 0), stop=(kt == KT - 1))

            for rnd in range(4):
                kpsum = kps.tile([P, 2048], F32, name="kpsum")
                for kt in range(KT):
                    for hh in range(4):
                        nc.tensor.matmul(
                            kpsum[:, hh * 512:hh * 512 + M],
                            wk_t[rnd][:, kt * 512 + hh * P:
                                      kt * 512 + (hh + 1) * P],
                            mem_sb[:, kt, :],
                            start=(kt == 0), stop=(kt == KT - 1))
                for hh in range(4):
                    nc.scalar.copy(kT_sb[:, rnd * 4 + hh, :],
                                   kpsum[:, hh * 512:hh * 512 + M])
                emit_ssq(range(rnd * 4, rnd * 4 + 4))

            s_sb = sp.tile([1, R], F32)
            nc.scalar.activation(
                s_sb, ssq[0:1, :], mybir.ActivationFunctionType.Sqrt,
                bias=eps_sb, scale=1.0 / H)
            nc.vector.reciprocal_approx_fast(s_sb, s_sb)
            nc.gpsimd.dma_start(out=rs_scr[:], in_=s_sb[0:1, :])


        # V proj after K/ssq psum pools close
        with tc.tile_pool(name="vps", bufs=2, space="PSUM") as vps:
            for dc in range(4):
                vpsum = vps.tile([P, 1024], F32, name="vpsum")
                for kt in range(KT):
                    for mt in range(MT):
                        nc.tensor.matmul(
                            vpsum[:, mt * 512:(mt + 1) * 512],
                            mem_sb[:, kt, mt * P:(mt + 1) * P],
                            wv_t[dc][:, kt * 512:(kt + 1) * 512],
                            start=(kt == 0), stop=(kt == KT - 1))
                for mt in range(MT):
                    nc.scalar.copy(vmd_sb[:, mt, dc * 512:(dc + 1) * 512],
                                   vpsum[:, mt * 512:(mt + 1) * 512])

        phaseA.close()

        # prefetch first two Q weight tiles (ahead of the blocking s_bc wait)
        wts = {}
        for u in (0, 1):
            wts[u] = wst.tile([P, KT * 256], BF16, name="wt")
            nc.sync.dma_start(out=wts[u], in_=wqTb[u])

        nc.gpsimd.dma_start(out=s_bc, in_=_bcast_ap(rs_scr[:]))
        for kt in range(KT):
            nc.vector.tensor_mul(xbf[:, kt, :], xbf[:, kt, :], s_bc)

        # ============ Main interleaved units ============
        with tc.tile_pool(name="pps", bufs=2, space="PSUM") as pps, \
             tc.tile_pool(name="gstg", bufs=2) as gstg:

            def emit_proj(u):
                htp = u % 8
                if u + 2 < 16:
                    wts[u + 2] = wst.tile([P, KT * 256], BF16, name="wt")
                    nc.sync.dma_start(out=wts[u + 2],
                                      in_=(wqTb if u + 2 < 8 else wgTb)[
                                          (u + 2) % 8])
                wt = wts.pop(u)
                if u < 8:
                    qt = qp.tile([P, 2, R], BF16, name="qt")
                    q_tiles[htp] = qt
                for h2 in range(2):
                    for rh in range(LH):
                        ppsum = pps.tile([P, 1024], F32, name="ppsum")
                        for kt in range(KT):
                            for lq in range(2):
                                sl = slice(rh * 1024 + lq * 512,
                                           rh * 1024 + (lq + 1) * 512)
                                nc.tensor.matmul(
                                    ppsum[:, lq * 512:(lq + 1) * 512],
                                    wt[:, kt * 256 + h2 * P:
                                       kt * 256 + (h2 + 1) * P],
                                    xbf[:, kt, sl],
                                    start=(kt == 0), stop=(kt == KT - 1))
                        osl = slice(rh * 1024, (rh + 1) * 1024)
                        if u < 8:
                            nc.scalar.copy(q_tiles[htp][:, h2, osl], ppsum)
                        else:
                            g = gstg.tile([P, 1024], BF16, name="g")
                            nc.scalar.activation(
                                g, ppsum,
                                mybir.ActivationFunctionType.Sigmoid)
                            nc.sync.dma_start(
                                out=gspill[htp * 2 + h2][:, osl], in_=g)

            with tc.tile_pool(name="qp", bufs=2) as qp, \
                 tc.tile_pool(name="sps", bufs=1, space="PSUM") as sps, \
                 tc.tile_pool(name="dps", bufs=1, space="PSUM") as dps, \
                 tc.tile_pool(name="aps", bufs=1, space="PSUM") as aps, \
                 tc.tile_pool(name="probs", bufs=3) as probsp, \
                 tc.tile_pool(name="aup", bufs=4) as aup, \
                 tc.tile_pool(name="asb", bufs=2) as asbp, \
                 tc.tile_pool(name="rdp", bufs=1) as rdp, \
                 tc.tile_pool(name="rbc", bufs=2) as rbcp:

                q_tiles = {}
                probs_t = {}
                au_t = {}
                rbc_t = {}

                def emit_scores_exp(pair):
                    qt = q_tiles[pair]
                    for hh in range(2):
                        h = pair * 2 + hh
                        pr = probsp.tile([P, MT, R], BF16, name="probs")
                        probs_t[h] = pr
                        for mt in range(MT):
                            for lh in range(LH):
                                spsum = sps.tile([P, 1024], F32, name="spsum")
                                for j in range(2):
                                    sl = slice((lh * 2 + j) * 512,
                                               (lh * 2 + j + 1) * 512)
                                    nc.tensor.matmul(
                                        spsum[:, j * 512:(j + 1) * 512],
                                        kT_sb[:, h, mt * P:(mt + 1) * P],
                                        qt[:, hh, sl], start=True, stop=True)
                                nc.scalar.activation(
                                    pr[:, mt, lh * 1024:(lh + 1) * 1024],
                                    spsum, mybir.ActivationFunctionType.Exp,
                                    bias=mask_sb[:, mt:mt + 1], scale=_SCALE)

                def emit_denom_attn(pair):
                    for hh in range(2):
                        h = pair * 2 + hh
                        pr = probs_t[h]
                        rden = rdp.tile([1, R], F32, name="rden")
                        for lq in range(LQ):
                            sl = slice(lq * 512, (lq + 1) * 512)
                            dpsum = dps.tile([1, 512], F32, name="dpsum")
                            for mt in range(MT):
                                nc.tensor.matmul(
                                    dpsum, ones_bf, pr[:, mt, sl],
                                    start=(mt == 0), stop=(mt == MT - 1))
                            nc.vector.reciprocal_approx_fast(rden[:, sl], dpsum)
                        nc.gpsimd.dma_start(out=rd_scr[h:h + 1, :],
                                            in_=rden[0:1, :])
                        rbc = rbcp.tile([P, R], BF16, name="rbc")
                        rbc_t[h] = rbc
                        nc.gpsimd.dma_start(out=rbc,
                                            in_=_bcast_ap(rd_scr[h, :]))
                        au = aup.tile([P, R], BF16, name="au")
                        au_t[h] = au
                        for lq in range(LQ):
                            sl = slice(lq * 512, (lq + 1) * 512)
                            apsum = aps.tile([P, 512], F32, name="apsum")
                            for mt in range(MT):
                                nc.tensor.matmul(
                                    apsum, vmd_sb[:, mt, h * P:(h + 1) * P],
                                    pr[:, mt, sl],
                                    start=(mt == 0), stop=(mt == MT - 1))
                            nc.scalar.copy(au[:, sl], apsum)
                        del probs_t[h]

                def emit_norm_spill(pair):
                    for hh in range(2):
                        h = pair * 2 + hh
                        attn_sb = asbp.tile([P, R], BF16, name="attn_sb")
                        nc.vector.tensor_mul(attn_sb, au_t[h], rbc_t[h])
                        nc.sync.dma_start(out=aspill[h], in_=attn_sb)
                        del au_t[h], rbc_t[h]

                for u in range(10):
                    pair = u - 1
                    if 0 <= pair < NHTP:
                        emit_scores_exp(pair)
                    emit_proj(u)
                    if 0 <= pair < NHTP:
                        emit_denom_attn(pair)
                        if pair >= 1:
                            emit_norm_spill(pair - 1)
                        if pair == NHTP - 1:
                            emit_norm_spill(pair)

            # attention pools closed: prefetch attn spills + first O weights
            at_stack = ExitStack()
            atp = at_stack.enter_context(tc.tile_pool(name="atp", bufs=1))
            at_sb = atp.tile([P, NH, R], BF16)
            wop = at_stack.enter_context(tc.tile_pool(name="wop", bufs=2))
            wo_ts = {}
            for i in (0, 1):
                wo_ts[i] = wop.tile([P, KT * 256], BF16, name="wo")
                nc.sync.dma_start(out=wo_ts[i], in_=woTb[i])
            for h in range(NH):
                nc.sync.dma_start(out=at_sb[:, h, :], in_=aspill[h])

            for u in range(10, 16):
                emit_proj(u)

            # ============ Phase D: O proj + gate ============
            with tc.tile_pool(name="gin", bufs=3) as ginp, \
                 tc.tile_pool(name="osb", bufs=2) as osbp:
                for htp in range(NHTP):
                    if htp + 2 < NHTP:
                        wo_ts[htp + 2] = wop.tile([P, KT * 256], BF16,
                                                  name="wo")
                        nc.sync.dma_start(out=wo_ts[htp + 2],
                                          in_=woTb[htp + 2])
                    wt = wo_ts.pop(htp)
                    for h2 in range(2):
                        o = htp * 2 + h2
                        for rh in range(LH):
                            osl = slice(rh * 1024, (rh + 1) * 1024)
                            g_in = ginp.tile([P, 1024], BF16, name="g_in")
                            nc.sync.dma_start(out=g_in,
                                              in_=gspill[o][:, osl])
                            o_sb = osbp.tile([P, 1024], F32, name="o_sb")
                            opsum = pps.tile([P, 1024], F32, name="ppsum")
                            for kt in range(KT):
                                for lq in range(2):
                                    sl = slice(rh * 1024 + lq * 512,
                                               rh * 1024 + (lq + 1) * 512)
                                    nc.tensor.matmul(
                                        opsum[:, lq * 512:(lq + 1) * 512],
                                        wt[:, kt * 256 + h2 * P:
                                           kt * 256 + (h2 + 1) * P],
                                        at_sb[:, kt, sl],
                                        start=(kt == 0), stop=(kt == KT - 1))
                            nc.vector.tensor_mul(o_sb, opsum, g_in)
                            nc.sync.dma_start(
                                out=outT[o * P:(o + 1) * P, osl], in_=o_sb)
            at_stack.close()

    nc.compile()
    return nc


# ===================== host side =====================

def _bf16(a):
    import ml_dtypes
    return np.ascontiguousarray(a.astype(ml_dtypes.bfloat16))


def _pack_w_256(w_io):
    # w_io [in=H, out=H] -> [NHTP, 128, KT*256]; [htp, p, kt*256+c] =
    # w_io[kt*128+p, htp*256+c]
    return _bf16(np.ascontiguousarray(
        w_io.reshape(_KT, P, _NHTP, 256).transpose(2, 1, 0, 3)
        .reshape(_NHTP, P, _KT * 256)))


def _pack_w_512(w_io):
    # w_io [in=H, out=H] -> [4, 128, KT*512]
    return _bf16(np.ascontiguousarray(
        w_io.reshape(_KT, P, 4, 512).transpose(2, 1, 0, 3)
        .reshape(4, P, _KT * 512)))


_nc_cache = {}


def kernel(hidden_states, memory_tokens, memory_mask, norm_w,
           wq, wk, wv, wo, wg):
    import concourse.bacc as bacc

    hs = np.asarray(hidden_states, dtype=np.float32)
    mem = np.asarray(memory_tokens, dtype=np.float32)
    mask = np.asarray(memory_mask)
    norm_w = np.asarray(norm_w, dtype=np.float32)

    wq_n = (np.asarray(wq, dtype=np.float32) * norm_w[None, :]).T
    wg_n = (np.asarray(wg, dtype=np.float32) * norm_w[None, :]).T
    shared = {
        "wqTb": _pack_w_256(np.ascontiguousarray(wq_n)),
        "wgTb": _pack_w_256(np.ascontiguousarray(wg_n)),
        "woTb": _pack_w_256(np.ascontiguousarray(
            np.asarray(wo, dtype=np.float32).T)),
        "wkTb": _pack_w_512(np.ascontiguousarray(
            np.asarray(wk, dtype=np.float32).T)),
        "wvTb": _pack_w_512(np.ascontiguousarray(
            np.asarray(wv, dtype=np.float32).T)),
    }

    # compact memory tokens: drop masked tokens, pad to MP
    cnt_max = int(mask.sum(axis=1).max())
    MP = 128 if cnt_max <= 128 else 256
    MTP = MP // P

    in_maps = []
    for c in range(_NCORES):
        b, half = c // 2, c % 2
        inp = dict(shared)
        hs_slice = hs[b, half * _R:(half + 1) * _R, :]
        inp["xTb"] = _bf16(np.ascontiguousarray(
            hs_slice.T.reshape(_KT, P, _R).transpose(1, 0, 2)
            .reshape(P, _KT * _R)))
        idx = np.nonzero(mask[b])[0]
        mem_c = np.zeros((MP, _H), dtype=np.float32)
        mem_c[:len(idx)] = mem[b][idx]
        mv = np.full(MP, -50.0, dtype=np.float32)
        mv[:len(idx)] = 0.0
        inp["memTb"] = _bf16(np.ascontiguousarray(
            mem_c.T.reshape(_KT, P, MP).transpose(1, 0, 2)
            .reshape(P, _KT * MP)))
        inp["maskb"] = np.ascontiguousarray(mv.reshape(MTP, P).T)
        in_maps.append(inp)

    if _nc_cache.get(MP) is None:
        nc = bacc.Bacc(None, target_bir_lowering=False, debug=False)
        build(nc, MP)
        _nc_cache[MP] = nc
    nc = _nc_cache[MP]

    import os
    trace = os.environ.get("KERNEL_TRACE") == "1"
    res = run_bass_kernel_spmd(nc, in_maps, core_ids=list(range(_NCORES)),
                               trace=trace)
    kernel.last_result = res

    out = np.empty((_B, _L, _H), dtype=np.float32)
    for c in range(_NCORES):
        b, half = c // 2, c % 2
        out[b, half * _R:(half + 1) * _R, :] = res.results[c]["outT"].T
    return out


# revision 18
# speedup vs baseline: 1.2253x; 1.0077x over previous
"""MemoryCrossAttention Trainium2 Bass kernel (v2, bf16 pipeline).

8-core data-parallel over query rows: core c handles batch c//2, row-half
c%2 (R=2048 rows). All matmuls run in bf16 (FWL weight loads, f32 PSUM
accumulation). Attention for head pair p is interleaved between projection
units (Q htp / G htp) so the PE stays dense and HAM-warm. PSUM budget:
4 banks proj (double-buffered groups) + 2 scores + 1 denom + 1 attn.
Softmax denominators use reciprocal_approx_fast + a DRAM stride-0
broadcast; attention output is normalized post-eviction on DVE.
"""
from contextlib import ExitStack

import numpy as np

import concourse.bass as bass
import concourse.tile as tile
from concourse import mybir
from concourse.bass_utils import run_bass_kernel_spmd

F32 = mybir.dt.float32
BF16 = mybir.dt.bfloat16
P = 128

_H, _NH, _HD, _M = 2048, 16, 128, 256
_B, _L = 4, 4096
_R = 2048            # rows per core
_NCORES = 8
_EPS = 1e-6
_KT = _H // P        # 16 contraction tiles
_MT = _M // P        # 2
_NHTP = _NH // 2     # 8 head pairs
_SCALE = _HD ** -0.5


def _bcast_ap(ap, p=P):
    return bass.AP(tensor=ap.tensor, offset=ap.offset, ap=[[0, p]] + ap.ap)


def build(nc, MP):
    H, NH, R, KT, NHTP = _H, _NH, _R, _KT, _NHTP
    M, MT = MP, MP // P
    LH = R // 1024       # 2 row-halves (1024) per R
    LQ = R // 512        # 4 512-chunks

    xTb = nc.dram_tensor("xTb", [P, KT * R], BF16, kind="ExternalInput")
    memTb = nc.dram_tensor("memTb", [P, KT * M], BF16, kind="ExternalInput")
    maskb = nc.dram_tensor("maskb", [P, MT], F32, kind="ExternalInput")
    wqTb = nc.dram_tensor("wqTb", [NHTP, P, KT * 256], BF16, kind="ExternalInput")
    wgTb = nc.dram_tensor("wgTb", [NHTP, P, KT * 256], BF16, kind="ExternalInput")
    woTb = nc.dram_tensor("woTb", [NHTP, P, KT * 256], BF16, kind="ExternalInput")
    wkTb = nc.dram_tensor("wkTb", [4, P, KT * 512], BF16, kind="ExternalInput")
    wvTb = nc.dram_tensor("wvTb", [4, P, KT * 512], BF16, kind="ExternalInput")
    outT = nc.dram_tensor("outT", [H, R], F32, kind="ExternalOutput")

    with tile.TileContext(nc) as tc, ExitStack() as ctx:
        dram = ctx.enter_context(tc.tile_pool(name="dram", bufs=1, space="DRAM"))
        aspill = dram.tile([NH, P, R], BF16)
        gspill = dram.tile([NH, P, R], BF16)
        rs_scr = dram.tile([R], BF16)
        rd_scr = dram.tile([NH, R], BF16)

        const = ctx.enter_context(tc.tile_pool(name="const", bufs=1))
        ones_f32 = const.tile([P, 1], F32)
        nc.vector.memset(ones_f32, 1.0)
        ones_bf = const.tile([P, 1], BF16)
        nc.vector.tensor_copy(ones_bf, ones_f32)
        eps_sb = const.tile([1, 1], F32)
        nc.vector.memset(eps_sb, _EPS)
        mask_sb = const.tile([P, MT], F32)

        # persistent SBUF tensors
        kv = ctx.enter_context(tc.tile_pool(name="kv", bufs=1))
        kT_sb = kv.tile([P, NH, M], BF16)       # [d, h, m]
        vmd_sb = kv.tile([P, MT, H], BF16)      # [m, mt, h*d]
        sbc_p = ctx.enter_context(tc.tile_pool(name="sbc", bufs=1))
        s_bc = sbc_p.tile([P, R], BF16)

        xbfp = ctx.enter_context(tc.tile_pool(name="xbfp", bufs=1))
        xbf = xbfp.tile([P, KT, R], BF16)

        wst = ctx.enter_context(tc.tile_pool(name="wst", bufs=3))

        phaseA = ExitStack()
        memp = phaseA.enter_context(tc.tile_pool(name="memp", bufs=1))
        mem_sb = memp.tile([P, KT, M], BF16)
        wkvp = phaseA.enter_context(tc.tile_pool(name="wkv", bufs=3))

        # ---- DMA issue order: wk0, mem, then x interleaved with weights ----
        wk_t, wv_t = {}, {}

        def alloc_wkv(d, i, src_t):
            d[i] = wkvp.tile([P, KT * 512], BF16, name="wkv")
            nc.sync.dma_start(out=d[i], in_=src_t[i])

        alloc_wkv(wk_t, 0, wkTb)
        nc.sync.dma_start(out=mem_sb, in_=memTb[:])
        nc.sync.dma_start(out=mask_sb, in_=maskb[:])
        x2p = phaseA.enter_context(tc.tile_pool(name="x2", bufs=2))
        x2s = []
        for kt in range(KT):
            nc.sync.dma_start(out=xbf[:, kt, :],
                              in_=xTb[:, kt * R:(kt + 1) * R])
            if kt in (3, 6, 9):
                alloc_wkv(wk_t, kt // 3, wkTb)
            if kt in (10, 11, 12, 13):
                alloc_wkv(wv_t, kt - 10, wvTb)
            x2 = x2p.tile([P, R], BF16, name="x2")
            nc.vector.tensor_mul(x2, xbf[:, kt, :], xbf[:, kt, :])
            x2s.append(x2)

        # ---- Phase A PE work: K rounds interleaved with ssq partition-sums --
        with tc.tile_pool(name="kps", bufs=1, space="PSUM") as kps, \
             tc.tile_pool(name="ssqp", bufs=1, space="PSUM") as ssqp, \
             tc.tile_pool(name="sp", bufs=1) as sp:
            ssq = ssqp.tile([1, R], F32)

            def emit_ssq(kts):
                for kt in kts:
                    for lq in range(LQ):
                        nc.tensor.matmul(
                            ssq[0:1, lq * 512:(lq + 1) * 512], ones_bf,
                            x2s[kt][:, lq * 512:(lq + 1) * 512],
                            start=(kt == 0), stop=(kt == KT - 1))

            ssq_plan = [range(0, 2), range(2, 6), range(6, 10), range(10, 14),
                        range(14, 16)]
            emit_ssq(ssq_plan[0])
            for rnd in range(4):
                kpsum = kps.tile([P, 2048], F32, name="kpsum")
                for kt in range(KT):
                    for hh in range(4):
                        nc.tensor.matmul(
                            kpsum[:, hh * 512:hh * 512 + M],
                            wk_t[rnd][:, kt * 512 + hh * P:
                                      kt * 512 + (hh + 1) * P],
                            mem_sb[:, kt, :],
                            start=(kt == 0), stop=(kt == KT - 1))
                for hh in range(4):
                    nc.scalar.copy(kT_sb[:, rnd * 4 + hh, :],
                                   kpsum[:, hh * 512:hh * 512 + M])
                emit_ssq(ssq_plan[rnd + 1])

            s_sb = sp.tile([1, R], F32)
            nc.scalar.activation(
                s_sb, ssq[0:1, :], mybir.ActivationFunctionType.Sqrt,
                bias=eps_sb, scale=1.0 / H)
            nc.vector.reciprocal_approx_fast(s_sb, s_sb)
            nc.gpsimd.dma_start(out=rs_scr[:], in_=s_sb[0:1, :])


        # V proj after K/ssq psum pools close
        with tc.tile_pool(name="vps", bufs=2, space="PSUM") as vps:
            for dc in range(4):
                vpsum = vps.tile([P, 1024], F32, name="vpsum")
                for kt in range(KT):
                    for mt in range(MT):
                        nc.tensor.matmul(
                            vpsum[:, mt * 512:(mt + 1) * 512],
                            mem_sb[:, kt, mt * P:(mt + 1) * P],
                            wv_t[dc][:, kt * 512:(kt + 1) * 512],
                            start=(kt == 0), stop=(kt == KT - 1))
                for mt in range(MT):
                    nc.scalar.copy(vmd_sb[:, mt, dc * 512:(dc + 1) * 512],
                                   vpsum[:, mt * 512:(mt + 1) * 512])

        phaseA.close()

        # prefetch first two Q weight tiles (ahead of the blocking s_bc wait)
        wts = {}
        for u in (0, 1):
            wts[u] = wst.tile([P, KT * 256], BF16, name="wt")
            nc.sync.dma_start(out=wts[u], in_=wqTb[u])

        nc.gpsimd.dma_start(out=s_bc, in_=_bcast_ap(rs_scr[:]))
        for kt in range(KT):
            nc.vector.tensor_mul(xbf[:, kt, :], xbf[:, kt, :], s_bc)

        # ============ Main interleaved units ============
        with tc.tile_pool(name="pps", bufs=2, space="PSUM") as pps, \
             tc.tile_pool(name="gstg", bufs=2) as gstg:

            def emit_proj(u):
                htp = u % 8
                if u + 2 < 16:
                    wts[u + 2] = wst.tile([P, KT * 256], BF16, name="wt")
                    nc.sync.dma_start(out=wts[u + 2],
                                      in_=(wqTb if u + 2 < 8 else wgTb)[
                                          (u + 2) % 8])
                wt = wts.pop(u)
                if u < 8:
                    qt = qp.tile([P, 2, R], BF16, name="qt")
                    q_tiles[htp] = qt
                for h2 in range(2):
                    for rh in range(LH):
                        ppsum = pps.tile([P, 1024], F32, name="ppsum")
                        for kt in range(KT):
                            for lq in range(2):
                                sl = slice(rh * 1024 + lq * 512,
                                           rh * 1024 + (lq + 1) * 512)
                                nc.tensor.matmul(
                                    ppsum[:, lq * 512:(lq + 1) * 512],
                                    wt[:, kt * 256 + h2 * P:
                                       kt * 256 + (h2 + 1) * P],
                                    xbf[:, kt, sl],
                                    start=(kt == 0), stop=(kt == KT - 1))
                        osl = slice(rh * 1024, (rh + 1) * 1024)
                        if u < 8:
                            nc.scalar.copy(q_tiles[htp][:, h2, osl], ppsum)
                        else:
                            g = gstg.tile([P, 1024], BF16, name="g")
                            nc.scalar.activation(
                                g, ppsum,
                                mybir.ActivationFunctionType.Sigmoid)
                            nc.sync.dma_start(
                                out=gspill[htp * 2 + h2][:, osl], in_=g)

            with tc.tile_pool(name="qp", bufs=2) as qp, \
                 tc.tile_pool(name="sps", bufs=1, space="PSUM") as sps, \
                 tc.tile_pool(name="dps", bufs=1, space="PSUM") as dps, \
                 tc.tile_pool(name="aps", bufs=1, space="PSUM") as aps, \
                 tc.tile_pool(name="probs", bufs=3) as probsp, \
                 tc.tile_pool(name="aup", bufs=4) as aup, \
                 tc.tile_pool(name="asb", bufs=2) as asbp, \
                 tc.tile_pool(name="rdp", bufs=1) as rdp, \
                 tc.tile_pool(name="rbc", bufs=2) as rbcp:

                q_tiles = {}
                probs_t = {}
                au_t = {}
                rbc_t = {}

                def emit_scores_exp(pair):
                    qt = q_tiles[pair]
                    for hh in range(2):
                        h = pair * 2 + hh
                        pr = probsp.tile([P, MT, R], BF16, name="probs")
                        probs_t[h] = pr
                        for mt in range(MT):
                            for lh in range(LH):
                                spsum = sps.tile([P, 1024], F32, name="spsum")
                                for j in range(2):
                                    sl = slice((lh * 2 + j) * 512,
                                               (lh * 2 + j + 1) * 512)
                                    nc.tensor.matmul(
                                        spsum[:, j * 512:(j + 1) * 512],
                                        kT_sb[:, h, mt * P:(mt + 1) * P],
                                        qt[:, hh, sl], start=True, stop=True)
                                nc.scalar.activation(
                                    pr[:, mt, lh * 1024:(lh + 1) * 1024],
                                    spsum, mybir.ActivationFunctionType.Exp,
                                    bias=mask_sb[:, mt:mt + 1], scale=_SCALE)

                def emit_denom_attn(pair):
                    for hh in range(2):
                        h = pair * 2 + hh
                        pr = probs_t[h]
                        rden = rdp.tile([1, R], F32, name="rden")
                        for lq in range(LQ):
                            sl = slice(lq * 512, (lq + 1) * 512)
                            dpsum = dps.tile([1, 512], F32, name="dpsum")
                            for mt in range(MT):
                                nc.tensor.matmul(
                                    dpsum, ones_bf, pr[:, mt, sl],
                                    start=(mt == 0), stop=(mt == MT - 1))
                            nc.vector.reciprocal_approx_fast(rden[:, sl], dpsum)
                        nc.gpsimd.dma_start(out=rd_scr[h:h + 1, :],
                                            in_=rden[0:1, :])
                        rbc = rbcp.tile([P, R], BF16, name="rbc")
                        rbc_t[h] = rbc
                        nc.gpsimd.dma_start(out=rbc,
                                            in_=_bcast_ap(rd_scr[h, :]))
                        au = aup.tile([P, R], BF16, name="au")
                        au_t[h] = au
                        for lq in range(LQ):
                            sl = slice(lq * 512, (lq + 1) * 512)
                            apsum = aps.tile([P, 512], F32, name="apsum")
                            for mt in range(MT):
                                nc.tensor.matmul(
                                    apsum, vmd_sb[:, mt, h * P:(h + 1) * P],
                                    pr[:, mt, sl],
                                    start=(mt == 0), stop=(mt == MT - 1))
                            nc.vector.tensor_copy(au[:, sl], apsum)
                        del probs_t[h]

                def emit_norm_spill(pair):
                    for hh in range(2):
                        h = pair * 2 + hh
                        attn_sb = asbp.tile([P, R], BF16, name="attn_sb")
                        nc.vector.tensor_mul(attn_sb, au_t[h], rbc_t[h])
                        nc.sync.dma_start(out=aspill[h], in_=attn_sb)
                        del au_t[h], rbc_t[h]

                for u in range(10):
                    pair = u - 1
                    if 0 <= pair < NHTP:
                        emit_scores_exp(pair)
                    emit_proj(u)
                    if 0 <= pair < NHTP:
                        emit_denom_attn(pair)
                        if pair >= 1:
                            emit_norm_spill(pair - 1)
                        if pair == NHTP - 1:
                            emit_norm_spill(pair)

            # attention pools closed: prefetch attn spills + first O weights
            at_stack = ExitStack()
            atp = at_stack.enter_context(tc.tile_pool(name="atp", bufs=1))
            at_sb = atp.tile([P, NH, R], BF16)
            wop = at_stack.enter_context(tc.tile_pool(name="wop", bufs=2))
            wo_ts = {}
            for i in (0, 1):
                wo_ts[i] = wop.tile([P, KT * 256], BF16, name="wo")
                nc.sync.dma_start(out=wo_ts[i], in_=woTb[i])
            for h in range(NH):
                nc.sync.dma_start(out=at_sb[:, h, :], in_=aspill[h])

            for u in range(10, 16):
                emit_proj(u)

            # ============ Phase D: O proj + gate ============
            with tc.tile_pool(name="gin", bufs=3) as ginp, \
                 tc.tile_pool(name="osb", bufs=2) as osbp:
                for htp in range(NHTP):
                    if htp + 2 < NHTP:
                        wo_ts[htp + 2] = wop.tile([P, KT * 256], BF16,
                                                  name="wo")
                        nc.sync.dma_start(out=wo_ts[htp + 2],
                                          in_=woTb[htp + 2])
                    wt = wo_ts.pop(htp)
                    for h2 in range(2):
                        o = htp * 2 + h2
                        for rh in range(LH):
                            osl = slice(rh * 1024, (rh + 1) * 1024)
                            g_in = ginp.tile([P, 1024], BF16, name="g_in")
                            nc.sync.dma_start(out=g_in,
                                              in_=gspill[o][:, osl])
                            o_sb = osbp.tile([P, 1024], F32, name="o_sb")
                            opsum = pps.tile([P, 1024], F32, name="ppsum")
                            for kt in range(KT):
                                for lq in range(2):
                                    sl = slice(rh * 1024 + lq * 512,
                                               rh * 1024 + (lq + 1) * 512)
                                    nc.tensor.matmul(
                                        opsum[:, lq * 512:(lq + 1) * 512],
                                        wt[:, kt * 256 + h2 * P:
                                           kt * 256 + (h2 + 1) * P],
                                        at_sb[:, kt, sl],
                                        start=(kt == 0), stop=(kt == KT - 1))
                            nc.vector.tensor_mul(o_sb, opsum, g_in)
                            nc.sync.dma_start(
                                out=outT[o * P:(o + 1) * P, osl], in_=o_sb)
            at_stack.close()

    nc.compile()
    return nc


# ===================== host side =====================

def _bf16(a):
    import ml_dtypes
    return np.ascontiguousarray(a.astype(ml_dtypes.bfloat16))


def _pack_w_256(w_io):
    # w_io [in=H, out=H] -> [NHTP, 128, KT*256]; [htp, p, kt*256+c] =
    # w_io[kt*128+p, htp*256+c]
    return _bf16(np.ascontiguousarray(
        w_io.reshape(_KT, P, _NHTP, 256).transpose(2, 1, 0, 3)
        .reshape(_NHTP, P, _KT * 256)))


def _pack_w_512(w_io):
    # w_io [in=H, out=H] -> [4, 128, KT*512]
    return _bf16(np.ascontiguousarray(
        w_io.reshape(_KT, P, 4, 512).transpose(2, 1, 0, 3)
        .reshape(4, P, _KT * 512)))


_nc_cache = {}


def kernel(hidden_states, memory_tokens, memory_mask, norm_w,
           wq, wk, wv, wo, wg):
    import concourse.bacc as bacc

    hs = np.asarray(hidden_states, dtype=np.float32)
    mem = np.asarray(memory_tokens, dtype=np.float32)
    mask = np.asarray(memory_mask)
    norm_w = np.asarray(norm_w, dtype=np.float32)

    wq_n = (np.asarray(wq, dtype=np.float32) * norm_w[None, :]).T
    wg_n = (np.asarray(wg, dtype=np.float32) * norm_w[None, :]).T
    shared = {
        "wqTb": _pack_w_256(np.ascontiguousarray(wq_n)),
        "wgTb": _pack_w_256(np.ascontiguousarray(wg_n)),
        "woTb": _pack_w_256(np.ascontiguousarray(
            np.asarray(wo, dtype=np.float32).T)),
        "wkTb": _pack_w_512(np.ascontiguousarray(
            np.asarray(wk, dtype=np.float32).T)),
        "wvTb": _pack_w_512(np.ascontiguousarray(
            np.asarray(wv, dtype=np.float32).T)),
    }

    # compact memory tokens: drop masked tokens, pad to MP
    cnt_max = int(mask.sum(axis=1).max())
    MP = 128 if cnt_max <= 128 else 256
    MTP = MP // P

    in_maps = []
    for c in range(_NCORES):
        b, half = c // 2, c % 2
        inp = dict(shared)
        hs_slice = hs[b, half * _R:(half + 1) * _R, :]
        inp["xTb"] = _bf16(np.ascontiguousarray(
            hs_slice.T.reshape(_KT, P, _R).transpose(1, 0, 2)
            .reshape(P, _KT * _R)))
        idx = np.nonzero(mask[b])[0]
        mem_c = np.zeros((MP, _H), dtype=np.float32)
        mem_c[:len(idx)] = mem[b][idx]
        mv = np.full(MP, -50.0, dtype=np.float32)
        mv[:len(idx)] = 0.0
        inp["memTb"] = _bf16(np.ascontiguousarray(
            mem_c.T.reshape(_KT, P, MP).transpose(1, 0, 2)
            .reshape(P, _KT * MP)))
        inp["maskb"] = np.ascontiguousarray(mv.reshape(MTP, P).T)
        in_maps.append(inp)

    if _nc_cache.get(MP) is None:
        nc = bacc.Bacc(None, target_bir_lowering=False, debug=False)
        build(nc, MP)
        _nc_cache[MP] = nc
    nc = _nc_cache[MP]

    import os
    trace = os.environ.get("KERNEL_TRACE") == "1"
    res = run_bass_kernel_spmd(nc, in_maps, core_ids=list(range(_NCORES)),
                               trace=trace)
    kernel.last_result = res

    out = np.empty((_B, _L, _H), dtype=np.float32)
    for c in range(_NCORES):
        b, half = c // 2, c % 2
        out[b, half * _R:(half + 1) * _R, :] = res.results[c]["outT"].T
    return out
